# revision 8
# baseline (speedup 1.0000x reference)
"""Trainium2 Bass kernel for nn_BrightnessImportanceSampler.

Reference semantics (B=32768 rays, S=512 spots):
    u            = jax.random.uniform(key(42), (B, S, 3))      # fixed constant
    num_ele      = bright_mask.sum(1)   (prefix mask, <= 256)
    num_ray      = ray_mask.sum(1)      (prefix mask, >= 256)
    jit          = spots[None] + std * u
    cond[b, j]   = (j < num_ele[b]) & (dot(jit[b,j], N[b]) > 0)
    t            = num_ray[b] - 1 - j                           # reversed write
    Ls[b, t]     = jit[b, j]   where cond                       # else 0
    bmask[b, t]  = cond[b, j]

Device formulation (per row b): since j < num_ele <= 256, only u[:, :256]
matters.  Work in m = 255 - j order (host pre-reverses u and spots), so the
valid block is already in ascending-t order:  t = num_ray - 256 + m.  Each row
then scatters one contiguous 768-float block (256 xyz granules) into the flat
output at element offset b*1536 + (num_ray-256)*3, plus a 256-byte block into
the flat bmask at b*512 + (num_ray-256).  The runtime pre-zeros ExternalOutput
buffers, so untouched regions are already 0.

Sharding: pure data parallel, B split across 8 cores (4096 rows each).
"""

import numpy as np
from contextlib import ExitStack

B, S, M = 32768, 512, 8
R = B // M            # 4096 rows per core
P = 128               # partitions
W = 768               # 256 granules * 3 floats
ROW_F = S * 3         # 1536 floats per output row

_cache = {}


# ----------------------------------------------------------------------------
# Tile framework fix: this container's walrus rejects instructions carrying
# more than one semaphore wait; TileContext's tail drain can carry several.
# Split them one-per-NOP before the drain.
# ----------------------------------------------------------------------------
def _make_tile_context_cls():
    import concourse.tile as tile
    from concourse import mybir
    from concourse.vector_clock import ScopedClock

    class TileContextSplitDrain(tile.TileContext):
        def _drain_and_barrier(self, tick_clock, wait_clock):
            nopi = self.nc.sync.nop(nofuse=True)
            wait_clock.add_sem_waits(
                nopi.ins, ScopedClock({None: tick_clock.global_clock})
            )
            si = nopi.ins.sync_info
            if si is not None and len(si.on_wait) > 1:
                waits = list(si.on_wait)
                si.on_wait = waits[:1]
                for w in waits[1:]:
                    n2 = self.nc.sync.nop(nofuse=True)
                    n2.ins.sync_info = mybir.SyncInfo(on_wait=[w], on_update=[])
            self.nc.sync.drain()
            self.nc.all_engine_barrier()
            assert self.sems is not None
            popped = self.nc._tile_sem_poison_stack.pop()
            assert popped is self._sem_poison
            self.nc.clear_and_free_semaphores(list(self.sems.allocated().values()))
            self.nc.all_engine_barrier()

    return TileContextSplitDrain


# ----------------------------------------------------------------------------
# Device program
# ----------------------------------------------------------------------------
def build_nc(rows=R, n_cores=M, bufs=4, dbg=False):
    import concourse.bass as bass
    from concourse import mybir
    f32, u8, i32 = mybir.dt.float32, mybir.dt.uint8, mybir.dt.int32
    Alu = mybir.AluOpType
    Act = mybir.ActivationFunctionType

    tiles = rows // P
    TileCtx = _make_tile_context_cls()

    nc = bass.Bass("TRN2", target_bir_lowering=False, debug=False,
                   num_devices=n_cores)

    dbg_t = {}
    if dbg:
        for nm, w in [("d_nr", 1), ("d_ne", 1), ("d_ldn", 256),
                      ("d_cond", 256), ("d_v", W), ("d_idx", 1),
                      ("d_bidx", 1), ("d_jit", W)]:
            dbg_t[nm] = nc.dram_tensor(nm, [rows, w],
                                       i32 if nm in ("d_idx", "d_bidx") else f32,
                                       kind="ExternalOutput")

    urev = nc.dram_tensor("urev", [rows, W], f32, kind="ExternalInput")
    rm = nc.dram_tensor("rm", [rows, S], u8, kind="ExternalInput")
    bm = nc.dram_tensor("bm", [rows, S], u8, kind="ExternalInput")
    # N pre-packed on host to [128, tiles*3]: nvec[p, t*3+c] = N[t*128+p, c]
    nvec = nc.dram_tensor("nvec", [P, tiles * 3], f32, kind="ExternalInput")
    # constants: [0:768] spots_rev replicated, [768:1024] iota 0..255,
    # [1024] p*1536, [1025] p*512, [1026] std
    cst = nc.dram_tensor("cst", [P, 1027], f32, kind="ExternalInput")

    ls = nc.dram_tensor("Ls", [rows * ROW_F, 1], f32, kind="ExternalOutput")
    bq = nc.dram_tensor("bmq", [rows * S, 1], u8, kind="ExternalOutput")

    with ExitStack() as ctx:
        tc = ctx.enter_context(TileCtx(nc))
        const_pool = ctx.enter_context(tc.tile_pool(name="const", bufs=1))
        in_pool = ctx.enter_context(tc.tile_pool(name="in", bufs=bufs))
        mid_pool = ctx.enter_context(tc.tile_pool(name="mid", bufs=bufs))
        out_pool = ctx.enter_context(tc.tile_pool(name="out", bufs=bufs))

        cst_t = const_pool.tile([P, 1027], f32)
        nc.sync.dma_start(cst_t[:], cst[:])
        nv_t = const_pool.tile([P, tiles * 3], f32)
        nc.sync.dma_start(nv_t[:], nvec[:])

        spots_ap = cst_t[:, 0:W]
        iota_ap = cst_t[:, W:W + 256]
        pb1536 = cst_t[:, 1024:1025]
        pb512 = cst_t[:, 1025:1026]
        stdv = cst_t[:, 1026:1027]

        for i in range(tiles):
            rsl = slice(i * P, (i + 1) * P)

            rm_t = in_pool.tile([P, S], u8, tag="rm")
            nc.sync.dma_start(rm_t[:], rm[rsl, :])
            bm_t = in_pool.tile([P, S], u8, tag="bm")
            nc.sync.dma_start(bm_t[:], bm[rsl, :])
            u_t = in_pool.tile([P, W], f32, tag="u")
            nc.sync.dma_start(u_t[:], urev[rsl, :])

            # row sums -> num_ray, num_ele (ACT engine: copy with accum_out)
            scr = mid_pool.tile([P, S], f32, tag="scr")
            nr_s = mid_pool.tile([P, 1], f32, tag="nr")   # num_ray
            nc.scalar.activation(scr[:], rm_t[:], Act.Identity, accum_out=nr_s[:])
            ne_s = mid_pool.tile([P, 1], f32, tag="ne")   # num_ele
            nc.scalar.activation(scr[:], bm_t[:], Act.Identity, accum_out=ne_s[:])

            # jit = std*u + spots  (both already reversed in m = 255-j order)
            jit = mid_pool.tile([P, W], f32, tag="jit")
            nc.vector.scalar_tensor_tensor(
                jit[:], u_t[:], stdv, spots_ap, op0=Alu.mult, op1=Alu.add)

            j3 = jit[:].rearrange("p (j c) -> p j c", c=3)
            nvi = nv_t[:, 3 * i:3 * i + 3]

            # LdotN = jit_x*Nx + jit_y*Ny + jit_z*Nz
            ldn = mid_pool.tile([P, 256], f32, tag="ldn")
            tmp = mid_pool.tile([P, 256], f32, tag="tmp")
            nc.vector.tensor_scalar(tmp[:], j3[:, :, 0], nvi[:, 0:1], None,
                                    op0=Alu.mult)
            nc.vector.scalar_tensor_tensor(
                ldn[:], j3[:, :, 1], nvi[:, 1:2], tmp[:],
                op0=Alu.mult, op1=Alu.add)
            nc.vector.scalar_tensor_tensor(
                ldn[:], j3[:, :, 2], nvi[:, 2:3], ldn[:],
                op0=Alu.mult, op1=Alu.add)

            # active: m >= 256 - num_ele  <=>  iota >= 256 - 512*ne_a
            thr = mid_pool.tile([P, 1], f32, tag="thr")
            nc.vector.tensor_scalar(thr[:], ne_s[:], -1.0, 256.0,
                                    op0=Alu.mult, op1=Alu.add)
            act = mid_pool.tile([P, 256], f32, tag="act")
            nc.vector.tensor_scalar(act[:], iota_ap, thr[:], None,
                                    op0=Alu.is_ge)

            # cond = (ldn > 0) * active
            cond = mid_pool.tile([P, 256], f32, tag="cond")
            nc.vector.scalar_tensor_tensor(
                cond[:], ldn[:], 0.0, act[:], op0=Alu.is_gt, op1=Alu.mult)

            # V = jit * cond (cond broadcast x3 along granule)
            cond_ap = cond[:]
            cond3 = bass.AP(cond_ap.tensor, cond_ap.offset,
                            cond_ap.ap + [[0, 3]])
            v_t = out_pool.tile([P, W], f32, tag="v")
            nc.vector.tensor_tensor(
                v_t[:].rearrange("p (j c) -> p j c", c=3), j3, cond3,
                op=Alu.mult)

            # Vb = cond cast to u8
            vb_t = out_pool.tile([P, 256], u8, tag="vb")
            nc.scalar.copy(vb_t[:], cond[:])

            # scatter offsets
            # idx  = (b0+p)*1536 + (num_ray-256)*3 = 1536*512*nr_a + pb1536 + (i*128*1536 - 768)
            idxf = mid_pool.tile([P, 1], f32, tag="idxf")
            nc.vector.scalar_tensor_tensor(
                idxf[:], nr_s[:], 3.0, pb1536, op0=Alu.mult, op1=Alu.add)
            idx = mid_pool.tile([P, 1], i32, tag="idx")
            nc.vector.tensor_scalar(idx[:], idxf[:],
                                    float(i * P * ROW_F - W), None, op0=Alu.add)
            bidxf = mid_pool.tile([P, 1], f32, tag="bidxf")
            nc.vector.scalar_tensor_tensor(
                bidxf[:], nr_s[:], 1.0, pb512, op0=Alu.mult, op1=Alu.add)
            bidx = mid_pool.tile([P, 1], i32, tag="bidx")
            nc.vector.tensor_scalar(bidx[:], bidxf[:],
                                    float(i * P * S - 256), None, op0=Alu.add)

            if dbg:
                rsl_d = slice(i * P, (i + 1) * P)
                nc.sync.dma_start(dbg_t["d_nr"][rsl_d, :], nr_s[:])
                nc.sync.dma_start(dbg_t["d_ne"][rsl_d, :], ne_s[:])
                nc.sync.dma_start(dbg_t["d_ldn"][rsl_d, :], ldn[:])
                nc.sync.dma_start(dbg_t["d_cond"][rsl_d, :], cond[:])
                nc.sync.dma_start(dbg_t["d_v"][rsl_d, :], v_t[:])
                nc.sync.dma_start(dbg_t["d_idx"][rsl_d, :], idx[:])
                nc.sync.dma_start(dbg_t["d_bidx"][rsl_d, :], bidx[:])
                nc.sync.dma_start(dbg_t["d_jit"][rsl_d, :], jit[:])

            nc.gpsimd.indirect_dma_start(
                out=ls[:], out_offset=bass.IndirectOffsetOnAxis(
                    ap=idx[:, 0:1], axis=0),
                in_=v_t[:], in_offset=None)
            nc.gpsimd.indirect_dma_start(
                out=bq[:], out_offset=bass.IndirectOffsetOnAxis(
                    ap=bidx[:, 0:1], axis=0),
                in_=vb_t[:], in_offset=None)

    return _split_multi_waits(nc)


def _split_multi_waits(nc):
    """This container's walrus rejects instructions carrying more than one
    semaphore wait.  Hoist extra waits onto same-engine NOPs placed just
    before the instruction (engines are in-order, so semantics are kept)."""
    from concourse import mybir
    for fn in nc.m.functions:
        for blk in fn.blocks:
            out = []
            for inst in blk.instructions:
                si = getattr(inst, "sync_info", None)
                if si is not None and si.on_wait and len(si.on_wait) > 1:
                    waits = list(si.on_wait)
                    for w in waits[:-1]:
                        nop = mybir.InstNoOp(
                            name=nc.get_next_instruction_name(),
                            engine=inst.engine,
                            ins=[], outs=[],
                            bass_nofuse=True,
                        )
                        nop.sync_info = mybir.SyncInfo(on_wait=[w],
                                                       on_update=[])
                        nc.register_instruction(nop, overwrite=True)
                        out.append(nop)
                    si.on_wait = waits[-1:]
                out.append(inst)
            blk.instructions[:] = out
    return nc


def _get_nc(rows=R):
    key = ("nc", rows)
    if key not in _cache:
        _cache[key] = build_nc(rows)
    return _cache[key]


# ----------------------------------------------------------------------------
# Host side
# ----------------------------------------------------------------------------
def _get_u_full():
    if "u_full" not in _cache:
        import jax
        import jax.numpy as jnp
        with jax.default_device(jax.devices("cpu")[0]):
            u = np.asarray(jax.random.uniform(
                jax.random.key(42), (B, S, 3), dtype=jnp.float32))
        _cache["u_full"] = u
    return _cache["u_full"]


def _get_urev():
    if "urev" not in _cache:
        u = _get_u_full()
        _cache["urev"] = np.ascontiguousarray(u[:, 255::-1, :]).reshape(B, W)
    return _cache["urev"]


def _consts(spots, std):
    cst = np.zeros((P, 1027), np.float32)
    cst[:, 0:W] = spots[:256][::-1].reshape(1, W)
    cst[:, W:W + 256] = np.arange(256, dtype=np.float32)[None, :]
    cst[:, 1024] = np.arange(P, dtype=np.float32) * ROW_F
    cst[:, 1025] = np.arange(P, dtype=np.float32) * S
    cst[:, 1026] = np.float32(std)
    return cst


def _fallback(N, spots, ray_mask, bright_mask, std):
    """General-case host computation (reference replica); used only if the
    inputs violate the prefix-mask contract assumed by the device kernel."""
    u = _get_u_full()
    num_ele = bright_mask.sum(1).astype(np.int64)
    num_ray = ray_mask.sum(1).astype(np.int64)
    j = np.arange(S)
    active = j[None, :] < num_ele[:, None]
    jit = (spots[None, :, :] + np.float32(std) * u).astype(np.float32)
    ldn = np.einsum('bsd,bd->bs', jit, N).astype(np.float32)
    cond = active & (ldn > 0.0)
    t = num_ray[:, None] - 1 - j[None, :]
    ok = cond & (t >= 0) & (t < S)
    ls = np.zeros((B, S, 3), np.float32)
    bmask = np.zeros((B, S), bool)
    bi, ji = np.nonzero(ok)
    ls[bi, t[bi, ji]] = jit[bi, ji]
    bmask[bi, t[bi, ji]] = True
    return ls, bmask


def kernel(V=None, N=None, spots=None, ray_mask=None, bright_mask=None,
           std=None, **_unused):
    N = np.ascontiguousarray(np.asarray(N, np.float32))
    spots = np.ascontiguousarray(np.asarray(spots, np.float32))
    rm8 = np.ascontiguousarray(np.asarray(ray_mask)).view(np.uint8)
    bm8 = np.ascontiguousarray(np.asarray(bright_mask)).view(np.uint8)
    stdf = float(np.asarray(std))

    num_ray = rm8.sum(1, dtype=np.int64)
    num_ele = bm8.sum(1, dtype=np.int64)
    if num_ray.min() < 256 or num_ele.max() > 256:
        return _fallback(N, spots, np.asarray(ray_mask), np.asarray(bright_mask),
                         stdf)

    from concourse.bass_utils import run_bass_kernel_spmd

    urev = _get_urev()
    cst = _consts(spots, stdf)
    tiles = R // P

    in_maps = []
    for c in range(M):
        rs = slice(c * R, (c + 1) * R)
        nv = N[rs].reshape(tiles, P, 3).transpose(1, 0, 2).reshape(P, tiles * 3)
        in_maps.append({
            "urev": urev[rs],
            "rm": rm8[rs],
            "bm": bm8[rs],
            "nvec": np.ascontiguousarray(nv),
            "cst": cst,
        })

    _cache["last_in_maps"] = in_maps
    nc = _get_nc()
    res = run_bass_kernel_spmd(nc, in_maps, list(range(M)))

    ls = np.concatenate(
        [res.results[c]["Ls"].reshape(R, S, 3) for c in range(M)], axis=0)
    bmask = np.concatenate(
        [res.results[c]["bmq"].reshape(R, S) for c in range(M)],
        axis=0).astype(bool)
    return ls, bmask
